# revision 1
# baseline (speedup 1.0000x reference)
"""2D Haar DWT (analysis) on 8 Trainium2 NeuronCores.

Input  x: (16, 64, 256, 256) f32  -> 1024 independent 256x256 images.
Output: tuple (LL, LH, HL, HH), each (16, 64, 128, 128) f32.

With Haar filters the DWT is a 2x2 butterfly: for each 2x2 block
(a b / c d), with the 0.5 scale folded into a host-side prescale:
    LL = a+b+c+d, LH = a-b+c-d, HL = a+b-c-d, HH = a-b-c+d
i.e. two levels of adds/subs -- no matmul. fp32 matmuls stream at half
rate on the PE and would dominate (measured 505us); plain VectorE adds
finish in ~145us per core, under the ~160us DMA-fabric floor for 67MB
of HBM traffic, so the kernel runs at the DMA roofline.

Layout (everything unit-stride, partition dim = image index):
  - host prescales x by 0.5 and deinterleaves even/odd columns
  - per core 128 images; rows processed in chunks; each chunk is one
    fully contiguous DRAM block [img, hc rows] so DMA descriptors are
    maximal (2MB transfers, 16KB/partition runs)
  - per chunk: one input DMA, 6 VectorE tensor ops, one output DMA.

Measured on hardware (neuron-profile, core 0): best 168,936 ns; fast
mode ~169-171us = 8.6us NRT preamble + 157.6us port-saturated DMA
stream (100% packed, ~27 GB/s x 16 engines) + 3.3us postamble.
~193-218us during episodic external contention on DMA engine 15.
For reference: naive HBM roofline ~187us; matmul formulation 505us.
"""

import numpy as np

import concourse.bacc as bacc
import concourse.tile as tile
from concourse import mybir
from concourse.bass_utils import run_bass_kernel_spmd

N_CORES = 8
B, C, H, W = 16, 64, 256, 256
N_IMG = B * C                    # 1024
P = N_IMG // N_CORES             # 128 images per core = partition dim
Wh = W // 2                      # 128
HC_BIG, N_BIG = 16, 16           # 16 compute chunks of 16 rows
IN_FACTOR = 1                    # input DMA granularity = 1 chunk (2MB);
                                 # 2-chunk/4MB DMAs measured +9us (DMA is
                                 # already at port rate; coarser buffers
                                 # just cost pipelining)
XP_BUFS = 5                      # 5 x 16KB/partition input buffers
assert HC_BIG * N_BIG == H and N_BIG % IN_FACTOR == 0
F32 = mybir.dt.float32

_CACHE = {}


def _butterfly(nc, xt, mid, op, hc):
    """Emit the 6 VectorE ops for one chunk; returns the output tile."""
    xv = xt.rearrange("p (h e w) -> p h e w", h=hc, e=2, w=Wh)
    xe = xv[:, :, 0, :].rearrange("p (i f) w -> p i f w", f=2)
    xo = xv[:, :, 1, :].rearrange("p (i f) w -> p i f w", f=2)
    sw = mid.tile([P, hc // 2, 2, Wh], F32, tag="sw")
    dw = mid.tile([P, hc // 2, 2, Wh], F32, tag="dw")
    nc.vector.tensor_add(sw, xe, xo)
    nc.vector.tensor_sub(dw, xe, xo)
    ot = op.tile([P, 4 * (hc // 2) * Wh], F32, tag="ot")
    ov = ot.rearrange("p (b i w) -> p b i w", b=4, i=hc // 2, w=Wh)
    nc.vector.tensor_add(ov[:, 0], sw[:, :, 0, :], sw[:, :, 1, :])  # LL
    nc.vector.tensor_add(ov[:, 1], dw[:, :, 0, :], dw[:, :, 1, :])  # LH
    nc.vector.tensor_sub(ov[:, 2], sw[:, :, 0, :], sw[:, :, 1, :])  # HL
    nc.vector.tensor_sub(ov[:, 3], dw[:, :, 0, :], dw[:, :, 1, :])  # HH
    return ot


def _build_program():
    nc = bacc.Bacc(
        "TRN2",
        target_bir_lowering=False,
        debug=False,
        enable_asserts=False,
        num_devices=N_CORES,
    )
    # input DMAs move IN_FACTOR compute-chunks at once (bigger descriptors,
    # fewer chunk boundaries); compute + output stay at HC_BIG granularity
    n_in = N_BIG // IN_FACTOR
    xb = nc.dram_tensor(
        "xb", [n_in, P, IN_FACTOR * HC_BIG * W], F32, kind="ExternalInput").ap()
    ob = nc.dram_tensor("ob", [N_BIG, P, HC_BIG * W], F32, kind="ExternalOutput").ap()

    with tile.TileContext(nc) as tc:
        with (
            tc.tile_pool(name="xp", bufs=XP_BUFS) as xp,
            tc.tile_pool(name="mid", bufs=3) as mid,
            tc.tile_pool(name="op", bufs=4) as op,
        ):
            csz = HC_BIG * W
            for k in range(n_in):
                xt = xp.tile([P, IN_FACTOR * csz], F32, tag="xt")
                nc.sync.dma_start(out=xt, in_=xb[k])
                for h in range(IN_FACTOR):
                    cid = k * IN_FACTOR + h
                    xc = xt[:, h * csz:(h + 1) * csz]
                    if cid < N_BIG - 1:
                        ot = _butterfly(nc, xc, mid, op, HC_BIG)
                        nc.scalar.dma_start(out=ob[cid], in_=ot)
                    else:
                        # final chunk: butterfly+store in 8-row halves so the
                        # first half's output overlaps the second half's
                        # compute, trimming the pipeline drain
                        hq = HC_BIG // 2
                        obv = ob[cid].rearrange(
                            "p (b i w) -> p b i w", b=4, i=HC_BIG // 2, w=Wh)
                        for q in range(2):
                            oth = _butterfly(
                                nc, xc[:, q * csz // 2:(q + 1) * csz // 2],
                                mid, op, hq)
                            othv = oth.rearrange(
                                "p (b i w) -> p b i w", b=4, i=hq // 2, w=Wh)
                            nc.scalar.dma_start(
                                out=obv[:, :, q * (hq // 2):(q + 1) * (hq // 2), :],
                                in_=othv)
    nc.compile()
    return nc


def kernel(x, m_l0, m_l1, m_h0, m_h1):
    x = np.asarray(x, dtype=np.float32)
    assert x.shape == (B, C, H, W), x.shape

    if "nc" not in _CACHE:
        _CACHE["nc"] = _build_program()
    nc = _CACHE["nc"]

    # prescale by 0.5 (exact) and split even/odd columns: [N, H, 2, W/2]
    xsp = (x.reshape(N_IMG, H, W // 2, 2) * np.float32(0.5)).transpose(0, 1, 3, 2)
    n_in = N_BIG // IN_FACTOR
    in_maps = []
    for s in range(N_CORES):
        shard = xsp[s * P:(s + 1) * P]  # [128, 256, 2, 128]
        big = shard.reshape(P, n_in, IN_FACTOR * HC_BIG * W).transpose(1, 0, 2)
        in_maps.append({"xb": np.ascontiguousarray(big)})

    res = run_bass_kernel_spmd(nc, in_maps, core_ids=list(range(N_CORES)))

    parts = []
    for s in range(N_CORES):
        obig = res.results[s]["ob"].reshape(N_BIG, P, 4, HC_BIG // 2, Wh)
        img = obig.transpose(1, 2, 0, 3, 4).reshape(P, 4, H // 2, Wh)
        parts.append(img)
    full = np.concatenate(parts, axis=0).reshape(B, C, 4, H // 2, Wh)
    LL = np.ascontiguousarray(full[:, :, 0])
    LH = np.ascontiguousarray(full[:, :, 1])
    HL = np.ascontiguousarray(full[:, :, 2])
    HH = np.ascontiguousarray(full[:, :, 3])
    return (LL, LH, HL, HH)



# revision 4
# speedup vs baseline: 1.8640x; 1.8640x over previous
"""2D Haar DWT (analysis) on 8 Trainium2 NeuronCores — fp16 I/O.

Input  x: (16, 64, 256, 256) f32  -> 1024 independent 256x256 images.
Output: tuple (LL, LH, HL, HH), each (16, 64, 128, 128) f32.

With Haar filters the DWT is a 2x2 butterfly: for each 2x2 block
(a b / c d), with the 0.5 scale folded into a host-side prescale:
    LL = a+b+c+d, LH = a-b+c-d, HL = a+b-c-d, HH = a-b-c+d
i.e. two levels of adds/subs -- no matmul.

The f32 version of this kernel is DMA-bound: 67MB of HBM traffic per
core streams at the ~425GB/s SBUF-AXI port rate -> ~158us + ~12us
NRT overhead. Moving device I/O to fp16 halves the bytes (DMA floor
~79us) and fp16 tensor_tensor runs in the DVE 2x perf mode, halving
vector time to ~71us — still just under the DMA floor. fp16 rounding
(input quant + two add stages + output quant) costs ~2.5e-4 l2 rel
error, well under the 2e-2 gate.

Layout: host prescales by 0.5, converts to fp16, and rearranges each
32-row chunk as [e(col parity), f(row parity), i(row pair), w] so all
six butterfly ops on the device are flat unit-stride slices (the DVE
2x mode requires step=1, 4B-aligned APs):
    s_e = a+c, d_e = a-c, s_o = b+d, d_o = b-d          (row stage)
    LL = s_e+s_o, LH = s_e-s_o, HL = d_e+d_o, HH = d_e-d_o (col stage)
Per chunk: one 2MB input DMA, 8 flat VectorE ops, one 2MB output DMA.
"""

import numpy as np

import concourse.bacc as bacc
import concourse.tile as tile
from concourse import mybir
from concourse.bass_utils import run_bass_kernel_spmd

N_CORES = 8
B, C, H, W = 16, 64, 256, 256
N_IMG = B * C                    # 1024
P = N_IMG // N_CORES             # 128 images per core = partition dim
Wh = W // 2                      # 128
HC, NCH = 32, 8                  # 8 chunks of 32 rows
IH = HC // 2                     # 16 row-pairs per chunk
CSZ = HC * W                     # 8192 elems / partition / chunk
QSZ = CSZ // 4                   # 2048 elems per (e,f) quadrant
XP_BUFS = 5
assert HC * NCH == H
F16 = mybir.dt.float16

_CACHE = {}


def _butterfly(nc, quads, mid, op, ih):
    """8 VectorE ops for one chunk; quads = (a, c, b, d) APs, each
    ih*Wh elems per partition. Returns the output tile [P, 4*ih*Wh]
    laid out [band, ih, Wh]."""
    q = ih * Wh
    a, c, b, d = quads
    se = mid.tile([P, q], F16, tag="se")
    de = mid.tile([P, q], F16, tag="de")
    so = mid.tile([P, q], F16, tag="so")
    do = mid.tile([P, q], F16, tag="do")
    nc.vector.tensor_add(se, a, c)
    nc.vector.tensor_sub(de, a, c)
    nc.vector.tensor_add(so, b, d)
    nc.vector.tensor_sub(do, b, d)
    ot = op.tile([P, 4 * q], F16, tag="ot")
    nc.vector.tensor_add(ot[:, 0 * q:1 * q], se, so)  # LL
    nc.vector.tensor_sub(ot[:, 1 * q:2 * q], se, so)  # LH
    nc.vector.tensor_add(ot[:, 2 * q:3 * q], de, do)  # HL
    nc.vector.tensor_sub(ot[:, 3 * q:4 * q], de, do)  # HH
    return ot


def _build_program():
    nc = bacc.Bacc(
        "TRN2",
        target_bir_lowering=False,
        debug=False,
        enable_asserts=False,
        num_devices=N_CORES,
    )
    xb = nc.dram_tensor("xb", [NCH, P, CSZ], F16, kind="ExternalInput").ap()
    ob = nc.dram_tensor("ob", [NCH, P, CSZ], F16, kind="ExternalOutput").ap()

    with tile.TileContext(nc) as tc:
        with (
            tc.tile_pool(name="xp", bufs=XP_BUFS) as xp,
            tc.tile_pool(name="mid", bufs=3) as mid,
            tc.tile_pool(name="op", bufs=4) as op,
        ):
            for k in range(NCH):
                xt = xp.tile([P, CSZ], F16, tag="xt")
                nc.sync.dma_start(out=xt, in_=xb[k])
                if k < NCH - 1:
                    quads = tuple(
                        xt[:, j * QSZ:(j + 1) * QSZ] for j in range(4))
                    ot = _butterfly(nc, quads, mid, op, IH)
                    nc.scalar.dma_start(out=ob[k], in_=ot)
                else:
                    # final chunk in 16-row halves so the first half's store
                    # overlaps the second half's compute (shorter drain)
                    hq = IH // 2  # 8 row-pairs per half
                    obv = ob[k].rearrange("p (b i w) -> p b i w", b=4, i=IH, w=Wh)
                    xv = xt.rearrange("p (q i w) -> p q i w", q=4, i=IH, w=Wh)
                    for s in range(2):
                        quads = tuple(
                            xv[:, j, s * hq:(s + 1) * hq, :] for j in range(4))
                        oth = _butterfly(nc, quads, mid, op, hq)
                        othv = oth.rearrange(
                            "p (b i w) -> p b i w", b=4, i=hq, w=Wh)
                        nc.scalar.dma_start(
                            out=obv[:, :, s * hq:(s + 1) * hq, :], in_=othv)
    nc.compile()
    return nc


def kernel(x, m_l0, m_l1, m_h0, m_h1):
    x = np.asarray(x, dtype=np.float32)
    assert x.shape == (B, C, H, W), x.shape

    if "nc" not in _CACHE:
        _CACHE["nc"] = _build_program()
    nc = _CACHE["nc"]

    # prescale by 0.5 (exact), quantize to fp16, and rearrange to
    # [chunk, img, e, f, i, w]
    x16 = (x.reshape(N_IMG, H, W) * np.float32(0.5)).astype(np.float16)
    xr = x16.reshape(N_IMG, NCH, IH, 2, Wh, 2)       # [n, ch, i, f, w, e]
    in_maps = []
    for s in range(N_CORES):
        shard = xr[s * P:(s + 1) * P]                 # [P, ch, i, f, w, e]
        big = shard.transpose(1, 0, 5, 3, 2, 4).reshape(NCH, P, CSZ)
        in_maps.append({"xb": np.ascontiguousarray(big)})

    res = run_bass_kernel_spmd(nc, in_maps, core_ids=list(range(N_CORES)))

    parts = []
    for s in range(N_CORES):
        obig = res.results[s]["ob"].reshape(NCH, P, 4, IH, Wh)
        img = obig.transpose(1, 2, 0, 3, 4).reshape(P, 4, H // 2, Wh)
        parts.append(img)
    full = np.concatenate(parts, axis=0).reshape(B, C, 4, H // 2, Wh)
    full = full.astype(np.float32)
    LL = np.ascontiguousarray(full[:, :, 0])
    LH = np.ascontiguousarray(full[:, :, 1])
    HL = np.ascontiguousarray(full[:, :, 2])
    HH = np.ascontiguousarray(full[:, :, 3])
    return (LL, LH, HL, HH)
